# revision 30
# baseline (speedup 1.0000x reference)
"""Trainium2 Bass kernel for nn_BERT4GCN_53884659695997.

Mathematical reduction
----------------------
In the reference, ``feature`` is reassigned to ``LN(guidance)`` at the top of
every loop iteration, so the GCN block's output is never consumed; only the
last BERT layer's branch (index 3 -> hidden_states layer 12, which skips the
GCN block) reaches the output:

    t[b]      = LN(relu(hs[12,b][ts[b]] @ guid_W[3] + guid_b[3])) * ln_g + ln_b
    logits[b] = ((t[b] * m[b,:,None]).sum(0) / m[b].sum(0)) @ cls_W + cls_b

(verified numerically against the jax reference).

Row gathers commute with the row-wise ops (matmul-by-row / relu / LN), so the
gather+mask folds into per-source-row weights w[r] = sum_i m[i]*[ts[i]==r].
Only rows with w[r] != 0 reach the output (~47 unique masked rows per
sample).  The host does the index bookkeeping: it collects each sample's
unique masked rows, packs them contiguously across the 8 samples of a core
(~375 rows -> padded to JB*128 columns), and LPT-balances samples across the
8 cores so every core fits the same JB.  The packed rows are staged
transposed ([768, JB*128], bf16) so the device consumes them directly as
matmul stationary operands.

Device math per core (all tensor arithmetic on device, bf16 operands with
fp32 PSUM accumulation; output tolerance is 2e-2, measured ~4e-3):

    G   = HST^T @ GW (+ guid_b via a ones-row matmul, when nonzero)   # PE
    GR  = relu(G)                  # ACT, PSUM -> SBUF
    mu, var = bn_stats/bn_aggr(GR) # DVE, one pass
    rs  = rsqrt(var + eps)         # ACT Abs_reciprocal_sqrt (relu's table)
    ASPT[h, s] = sum_j GRX[j, h] * (w_pre*rs*sel)[j, s]   # PE (col 600 = mu)
    logits^T   = CWG^T @ ASPT + CLSB                      # PE + DVE

LN folds into the classifier: the affine (ln_g, ln_b), the -mu correction
(via the extra mu column paired with a -sum(CWG) classifier row) and the
1/sum(m) normalization (folded into w_pre host-side) are all exact linear
algebra.  Sharding: data-parallel over batch B=64 -> 8 samples per core.

The repeat loop (measurement) unrolls 16 bodies inside a staggered-reset
tc.For_i with software-pipelined input loads: every body refills its HST
buffer right after its guidance matmuls consume it, so the loop back-edge
never waits on a DMA and the PE stream stays dense (HAM stays at full
clock).
"""

import numpy as np
import ml_dtypes
from contextlib import ExitStack

import concourse.bass as bass
import concourse.tile as tile
from concourse import bacc, mybir
from concourse.bass_utils import run_bass_kernel_spmd

F32 = mybir.dt.float32
BF16 = mybir.dt.bfloat16
AX = mybir.AxisListType
ALU = mybir.AluOpType
ACTF = mybir.ActivationFunctionType

N_CORES = 8
B = 64
BC = B // N_CORES
L = 256
D = 768
H = 600
KT = D // 128            # 6 contraction tiles
EPS = 1e-5
HCH = ((0, 128), (128, 256), (256, 384), (384, 512), (512, 601))
BF = ml_dtypes.bfloat16


def build_program(jb: int = 3, repeats: int = 1, has_bias: bool = False):
    jt = jb * 128
    nc = bacc.Bacc("TRN2", target_bir_lowering=False, debug=False,
                   num_devices=N_CORES)

    dr = {}
    def din(name, shape, dt=F32):
        dr[name] = nc.dram_tensor(name, list(shape), dt, kind="ExternalInput").ap()
    din("hst", (D, jt), BF16)        # packed gathered rows, transposed
    din("gw", (D, H), BF16)
    din("sels", (128, jb * BC))      # row-to-sample one-hot (packed layout)
    din("wpre", (128, jb))           # gather weights / sum(m), packed layout
    din("cwg", (640, 3))             # ln_g-folded cls_W; row 600 = -colsum
    din("clsb3", (3, BC))            # (ln_b @ cls_W + cls_b) replicated
    din("epsc", (128, 1))            # LN epsilon, ACT bias operand
    if has_bias:
        din("gbrow", (1, H), BF16)
        din("onesrow", (1, 128), BF16)
    out_ap = nc.dram_tensor("out", [3, BC], F32, kind="ExternalOutput").ap()

    with tile.TileContext(nc) as tc, ExitStack() as ctx:
        cpool = ctx.enter_context(tc.tile_pool(name="consts", bufs=1))
        hpool = ctx.enter_context(tc.tile_pool(name="stream", bufs=16))
        apool = ctx.enter_context(tc.tile_pool(name="act", bufs=2))
        stats = ctx.enter_context(tc.tile_pool(name="stats", bufs=2))
        pg_ps = ctx.enter_context(tc.tile_pool(name="pg", bufs=2, space="PSUM"))
        sm_ps = ctx.enter_context(tc.tile_pool(name="sm", bufs=2, space="PSUM"))

        # ---- constants (loaded once) ----
        GWS = cpool.tile([128, KT, H], BF16, tag="gws")
        nc.sync.dma_start(GWS[:], dr["gw"].rearrange("(kt p) n -> p kt n", p=128))
        SELS = cpool.tile([128, jb * BC], F32, tag="sels")
        nc.sync.dma_start(SELS[:], dr["sels"][:])
        WPRE = cpool.tile([128, jb], F32, tag="wpre")
        nc.sync.dma_start(WPRE[:], dr["wpre"][:])
        CWGS = cpool.tile([128, 5, 3], F32, tag="cwgs")
        nc.sync.dma_start(CWGS[:], dr["cwg"].rearrange("(c p) n -> p c n", p=128))
        CLSB3 = cpool.tile([3, BC], F32, tag="clsb3")
        nc.sync.dma_start(CLSB3[:], dr["clsb3"][:])
        EPSC = cpool.tile([128, 1], F32, tag="epsc")
        nc.sync.dma_start(EPSC[:], dr["epsc"][:])
        # dummy Sqrt before the loop pins the act-function table to
        # sqrt_and_others (contains Relu/Sqrt/Copy) so the in-loop Sqrt
        # never triggers a 1283ns table reload
        DUMS = cpool.tile([1, 1], F32, tag="dums")
        nc.scalar.activation(DUMS[:], EPSC[0:1, :], ACTF.Sqrt)
        if has_bias:
            GBROW = cpool.tile([1, H], BF16, tag="gbrow")
            nc.sync.dma_start(GBROW[:], dr["gbrow"][:])
            ONESR = cpool.tile([1, 128], BF16, tag="onesrow")
            nc.sync.dma_start(ONESR[:], dr["onesrow"][:])

        def load_hst():
            # one DMA for the packed gathered rows (the only big input)
            HSTS = hpool.tile([128, KT, jt], BF16, tag="hsts")
            refill_hst(HSTS)
            return HSTS

        def refill_hst(HSTS):
            nc.sync.dma_start(HSTS[:],
                              dr["hst"].rearrange("(kt p) j -> p kt j", p=128))

        def body(HSTS):
            MV6 = stats.tile([128, jb, 12], F32, tag="mv6")
            MV = stats.tile([128, jb, 2], F32, tag="mv")
            GRS = []
            for k in range(jb):
                ksl = slice(k * 128, (k + 1) * 128)
                # guidance matmul: out[j, n] accumulated over 6 k-tiles
                PGA = pg_ps.tile([128, 512], F32, tag="pga")
                PGB = pg_ps.tile([128, 88], F32, tag="pgb")
                if has_bias:
                    nc.tensor.matmul(PGA[:], ONESR[:], GBROW[:, 0:512],
                                     start=True, stop=False)
                    nc.tensor.matmul(PGB[:], ONESR[:], GBROW[:, 512:600],
                                     start=True, stop=False)
                for kt in range(KT):
                    st = (kt == 0) and not has_bias
                    sp = kt == KT - 1
                    nc.tensor.matmul(PGA[:], HSTS[:, kt, ksl], GWS[:, kt, 0:512],
                                     start=st, stop=sp)
                    nc.tensor.matmul(PGB[:], HSTS[:, kt, ksl], GWS[:, kt, 512:600],
                                     start=st, stop=sp)
                GR = apool.tile([128, 601], BF16, tag=f"gr{k}")
                nc.scalar.activation(GR[:, 0:512], PGA[:], ACTF.Relu)
                nc.scalar.activation(GR[:, 512:600], PGB[:], ACTF.Relu)
                # LN stats in one DVE pass (two equal 300-col chunks so
                # bn_aggr's unweighted combine is exact)
                nc.vector.bn_stats(MV6[:, k, 0:6], GR[:, 0:300])
                nc.vector.bn_stats(MV6[:, k, 6:12], GR[:, 300:600])
                nc.vector.bn_aggr(MV[:, k, :], MV6[:, k, :])
                # mu column pairs with the classifier's -colsum row
                nc.vector.tensor_copy(GR[:, 600:601], MV[:, k, 0:1])
                GRS.append(GR)

            # rs = rsqrt(var + eps) via ACT Sqrt + DVE reciprocal.  The
            # pre-loop dummy Sqrt keeps the act table pinned, so this costs
            # one ~90ns ACT op instead of a per-iteration table reload.
            SD = stats.tile([128, jb], F32, tag="sd")
            nc.scalar.activation(SD[:], MV[:, :, 1], ACTF.Sqrt, bias=EPSC[:])
            Y = stats.tile([128, jb], F32, tag="y")
            nc.vector.reciprocal(Y[:], SD[:])
            W2 = stats.tile([128, jb], F32, tag="w2")
            nc.vector.tensor_mul(W2[:], WPRE[:], Y[:])
            W2F = stats.tile([128, jb * BC], BF16, tag="w2f")
            for k in range(jb):
                nc.vector.tensor_scalar(W2F[:, k * BC:(k + 1) * BC],
                                        SELS[:, k * BC:(k + 1) * BC],
                                        W2[:, k:k + 1], None, ALU.mult)

            # ---- aspects^T [601, BC] then classifier [3, BC] ----
            ASPT = sm_ps.tile([128, 5, BC], F32, tag="aspt")
            for hc, (hlo, hhi) in enumerate(HCH):
                sz = hhi - hlo
                for k in range(jb):
                    nc.tensor.matmul(ASPT[:sz, hc, :], GRS[k][:, hlo:hhi],
                                     W2F[:, k * BC:(k + 1) * BC],
                                     start=(k == 0), stop=(k == jb - 1))
            ASB = stats.tile([128, 5, BC], F32, tag="asb")
            nc.scalar.copy(ASB[:, 0:4, :], ASPT[:, 0:4, :])
            nc.vector.tensor_copy(ASB[0:89, 4, :], ASPT[0:89, 4, :])
            LG = sm_ps.tile([3, BC], F32, tag="lg")
            for hc, (hlo, hhi) in enumerate(HCH):
                sz = hhi - hlo
                nc.tensor.matmul(LG[:], CWGS[:sz, hc, :], ASB[:sz, hc, :],
                                 start=(hc == 0), stop=(hc == 4))
            OSB = stats.tile([3, BC], F32, tag="osb")
            nc.vector.tensor_add(OSB[:], LG[:], CLSB3[:])
            # issue from ACT: keeps the in-order SP queue free so the next
            # iteration's HSTS load can issue as soon as its WAR clears
            nc.scalar.dma_start(out_ap[:], OSB[:])

        UNROLL = 16
        if repeats <= UNROLL:
            hs = [load_hst() for _ in range(repeats)]
            for u in range(repeats):
                body(hs[u])
        else:
            # software-pipelined loads: the prologue fills all 8 buffers;
            # each body refills its buffer for the NEXT loop iteration as
            # soon as its guidance matmuls have consumed it, so the loop
            # back-edge never waits on a DMA
            assert repeats % UNROLL == 0, f"repeat count must divide {UNROLL}"
            hs = [load_hst() for _ in range(UNROLL)]
            with tc.For_i(0, repeats // UNROLL, 1, staggered_reset=True):
                for u in range(UNROLL):
                    body(hs[u])
                    refill_hst(hs[u])

    nc.compile()
    return nc


def prepare(inputs):
    """Host-side prep: pure index bookkeeping (unique-row packing, sample->
    core balancing, one-hot/selection masks) plus exact linear-algebra folds
    of the constant parameters.  All data-scale tensor arithmetic stays on
    device."""
    hs12 = np.asarray(inputs["hidden_states"])[12]              # [B, L, D]
    ts = np.asarray(inputs["token_starts"]).astype(np.int64)
    m = np.asarray(inputs["aspect_in_text_mask"], dtype=np.float32)
    gw = np.asarray(inputs["guid_W"], dtype=np.float32)[3]      # [D, H]
    gb = np.asarray(inputs["guid_b"], dtype=np.float32)[3]
    ln_g = np.asarray(inputs["ln_g"], dtype=np.float32)
    ln_b = np.asarray(inputs["ln_b"], dtype=np.float32)
    cls_W = np.asarray(inputs["cls_W"], dtype=np.float32)
    cls_b = np.asarray(inputs["cls_b"], dtype=np.float32)

    used_rows = [np.unique(ts[b][m[b] > 0]) for b in range(B)]
    ju = np.array([len(u) for u in used_rows])
    # LPT-balance samples across cores (exactly BC samples per core)
    order = np.argsort(-ju, kind="stable")
    cores = [[] for _ in range(N_CORES)]
    loads = np.zeros(N_CORES, np.int64)
    for b in order:
        cands = [c for c in range(N_CORES) if len(cores[c]) < BC]
        c = min(cands, key=lambda c: (loads[c], len(cores[c])))
        cores[c].append(int(b))
        loads[c] += ju[b]
    jb = max(1, int(np.ceil(loads.max() / 128)))
    jt = jb * 128
    has_bias = bool(np.any(gb != 0.0))

    cwg = ln_g[:, None] * cls_W                                  # [600, 3]
    cwg_full = np.zeros((640, 3), np.float32)
    cwg_full[:H] = cwg
    cwg_full[600] = -cwg.sum(0)
    clsb3 = np.tile((ln_b @ cls_W + cls_b)[:, None], (1, BC)).astype(np.float32)
    gw_b = np.ascontiguousarray(gw).astype(BF)

    in_maps = []
    for c in range(N_CORES):
        hst = np.zeros((D, jt), np.float32)
        wpre_flat = np.zeros(jt, np.float32)
        sel_flat = np.zeros((jt, BC), np.float32)
        j = 0
        for si, b in enumerate(cores[c]):
            rows = used_rows[b]
            msk = m[b] > 0
            cnt = np.zeros(L, np.float32)
            np.add.at(cnt, ts[b][msk], m[b][msk])
            n = len(rows)
            hst[:, j:j + n] = hs12[b][rows].T
            wpre_flat[j:j + n] = cnt[rows] / m[b].sum()
            sel_flat[j:j + n, si] = 1.0
            j += n
        hst[:, j:] = hst[:, 0:1]          # pad with a real column (w=0)
        # packed j -> (p = j % 128, k = j // 128)
        wpre = wpre_flat.reshape(jb, 128).T.copy()
        sels = sel_flat.reshape(jb, 128, BC).transpose(1, 0, 2).reshape(128, jb * BC).copy()
        im = dict(
            hst=np.ascontiguousarray(hst).astype(BF),
            gw=gw_b,
            sels=sels,
            wpre=wpre,
            cwg=cwg_full,
            clsb3=clsb3,
            epsc=np.full((128, 1), EPS, np.float32),
        )
        if has_bias:
            im["gbrow"] = gb[None, :].astype(BF)
            im["onesrow"] = np.ones((1, 128), BF)
        in_maps.append(im)
    return in_maps, cores, jb, has_bias


_PROGRAMS = {}


def kernel(**inputs):
    in_maps, cores, jb, has_bias = prepare(inputs)
    key = (jb, has_bias)
    nc = _PROGRAMS.get(key)
    if nc is None:
        nc = _PROGRAMS[key] = build_program(jb=jb, repeats=1, has_bias=has_bias)
    res = run_bass_kernel_spmd(nc, in_maps, list(range(N_CORES)), trace=False)
    out = np.zeros((B, 3), np.float32)
    for c in range(N_CORES):
        oc = np.asarray(res.results[c]["out"])   # [3, BC]
        for si, b in enumerate(cores[c]):
            out[b] = oc[:, si]
    return out


# revision 33
# speedup vs baseline: 1.0289x; 1.0289x over previous
"""Trainium2 Bass kernel for nn_BERT4GCN_53884659695997.

Mathematical reduction
----------------------
In the reference, ``feature`` is reassigned to ``LN(guidance)`` at the top of
every loop iteration, so the GCN block's output is never consumed; only the
last BERT layer's branch (index 3 -> hidden_states layer 12, which skips the
GCN block) reaches the output:

    t[b]      = LN(relu(hs[12,b][ts[b]] @ guid_W[3] + guid_b[3])) * ln_g + ln_b
    logits[b] = ((t[b] * m[b,:,None]).sum(0) / m[b].sum(0)) @ cls_W + cls_b

(verified numerically against the jax reference).

Row gathers commute with the row-wise ops (matmul-by-row / relu / LN), so the
gather+mask folds into per-source-row weights w[r] = sum_i m[i]*[ts[i]==r].
Only rows with w[r] != 0 reach the output (~47 unique masked rows per
sample).  The host does the index bookkeeping: it collects each sample's
unique masked rows, packs them contiguously across the 8 samples of a core
(~375 rows -> padded to JB*128 columns), and LPT-balances samples across the
8 cores so every core fits the same JB.  The packed rows are staged
transposed ([768, JB*128], bf16) so the device consumes them directly as
matmul stationary operands.

Device math per core (all tensor arithmetic on device, bf16 operands with
fp32 PSUM accumulation; output tolerance is 2e-2, measured ~4e-3):

    G   = HST^T @ GW (+ guid_b via a ones-row matmul, when nonzero)   # PE
    GR  = relu(G)                  # ACT, PSUM -> SBUF
    mu, var = bn_stats/bn_aggr(GR) # DVE, one pass
    rs  = rsqrt(var + eps)         # ACT Sqrt (pinned table) + DVE recip
    ASPT[h, s] = sum_j GRX[j, h] * (w_pre*rs*sel)[j, s]   # PE (col 600 = mu)
    logits^T   = CWG^T @ ASPT + CLSB                      # PE + DVE

LN folds into the classifier: the affine (ln_g, ln_b), the -mu correction
(via the extra mu column paired with a -sum(CWG) classifier row) and the
1/sum(m) normalization (folded into w_pre host-side) are all exact linear
algebra.  Sharding: data-parallel over batch B=64 -> 8 samples per core.

The repeat loop (measurement) unrolls 16 bodies inside a staggered-reset
tc.For_i with software-pipelined input loads: every body refills its HST
buffer right after its guidance matmuls consume it, so the loop back-edge
never waits on a DMA and the PE stream stays dense (HAM stays at full
clock).
"""

import numpy as np
import ml_dtypes
from contextlib import ExitStack

import concourse.bass as bass
import concourse.tile as tile
from concourse import bacc, mybir
from concourse.bass_utils import run_bass_kernel_spmd

F32 = mybir.dt.float32
BF16 = mybir.dt.bfloat16
AX = mybir.AxisListType
ALU = mybir.AluOpType
ACTF = mybir.ActivationFunctionType

N_CORES = 8
B = 64
BC = B // N_CORES
L = 256
D = 768
H = 600
KT = D // 128            # 6 contraction tiles
EPS = 1e-5
HCH = ((0, 128), (128, 256), (256, 384), (384, 512), (512, 601))
BF = ml_dtypes.bfloat16


def build_program(jb: int = 3, repeats: int = 1, has_bias: bool = False):
    jt = jb * 128
    nc = bacc.Bacc("TRN2", target_bir_lowering=False, debug=False,
                   num_devices=N_CORES)

    dr = {}
    def din(name, shape, dt=F32):
        dr[name] = nc.dram_tensor(name, list(shape), dt, kind="ExternalInput").ap()
    din("hst", (D, jt), BF16)        # packed gathered rows, transposed
    din("gw", (D, H), BF16)
    din("sels", (128, jb * BC))      # row-to-sample one-hot (packed layout)
    din("wpre", (128, jb))           # gather weights / sum(m), packed layout
    din("cwg", (640, 3))             # ln_g-folded cls_W; row 600 = -colsum
    din("clsb3", (3, BC))            # (ln_b @ cls_W + cls_b) replicated
    din("epsc", (128, 1))            # LN epsilon, ACT bias operand
    if has_bias:
        din("gbrow", (1, H), BF16)
        din("onesrow", (1, 128), BF16)
    out_ap = nc.dram_tensor("out", [3, BC], F32, kind="ExternalOutput").ap()

    with tile.TileContext(nc) as tc, ExitStack() as ctx:
        cpool = ctx.enter_context(tc.tile_pool(name="consts", bufs=1))
        hpool = ctx.enter_context(tc.tile_pool(name="stream", bufs=32))
        apool = ctx.enter_context(tc.tile_pool(name="act", bufs=2))
        stats = ctx.enter_context(tc.tile_pool(name="stats", bufs=2))
        pg_ps = ctx.enter_context(tc.tile_pool(name="pg", bufs=3, space="PSUM"))
        sm_ps = ctx.enter_context(tc.tile_pool(name="sm", bufs=1, space="PSUM"))

        # ---- constants (loaded once) ----
        GWS = cpool.tile([128, KT, H], BF16, tag="gws")
        nc.sync.dma_start(GWS[:], dr["gw"].rearrange("(kt p) n -> p kt n", p=128))
        SELS = cpool.tile([128, jb * BC], F32, tag="sels")
        nc.sync.dma_start(SELS[:], dr["sels"][:])
        WPRE = cpool.tile([128, jb], F32, tag="wpre")
        nc.sync.dma_start(WPRE[:], dr["wpre"][:])
        CWGS = cpool.tile([128, 5, 3], F32, tag="cwgs")
        nc.sync.dma_start(CWGS[:], dr["cwg"].rearrange("(c p) n -> p c n", p=128))
        CLSB3 = cpool.tile([3, BC], F32, tag="clsb3")
        nc.sync.dma_start(CLSB3[:], dr["clsb3"][:])
        EPSC = cpool.tile([128, 1], F32, tag="epsc")
        nc.sync.dma_start(EPSC[:], dr["epsc"][:])
        # dummy Sqrt before the loop pins the act-function table to
        # sqrt_and_others (contains Relu/Sqrt/Copy) so the in-loop Sqrt
        # never triggers a 1283ns table reload
        DUMS = cpool.tile([1, 1], F32, tag="dums")
        nc.scalar.activation(DUMS[:], EPSC[0:1, :], ACTF.Sqrt)
        if has_bias:
            GBROW = cpool.tile([1, H], BF16, tag="gbrow")
            nc.sync.dma_start(GBROW[:], dr["gbrow"][:])
            ONESR = cpool.tile([1, 128], BF16, tag="onesrow")
            nc.sync.dma_start(ONESR[:], dr["onesrow"][:])

        def load_hst():
            # one DMA for the packed gathered rows (the only big input)
            HSTS = hpool.tile([128, KT, jt], BF16, tag="hsts")
            refill_hst(HSTS)
            return HSTS

        def refill_hst(HSTS):
            nc.sync.dma_start(HSTS[:],
                              dr["hst"].rearrange("(kt p) j -> p kt j", p=128))

        def body(HSTS):
            MV6 = stats.tile([128, jb, 12], F32, tag="mv6")
            MV = stats.tile([128, jb, 2], F32, tag="mv")
            GRS = []
            for k in range(jb):
                ksl = slice(k * 128, (k + 1) * 128)
                # guidance matmul: out[j, n] accumulated over 6 k-tiles
                PGA = pg_ps.tile([128, 512], F32, tag="pga")
                PGB = pg_ps.tile([128, 88], F32, tag="pgb")
                if has_bias:
                    nc.tensor.matmul(PGA[:], ONESR[:], GBROW[:, 0:512],
                                     start=True, stop=False)
                    nc.tensor.matmul(PGB[:], ONESR[:], GBROW[:, 512:600],
                                     start=True, stop=False)
                for kt in range(KT):
                    st = (kt == 0) and not has_bias
                    sp = kt == KT - 1
                    nc.tensor.matmul(PGA[:], HSTS[:, kt, ksl], GWS[:, kt, 0:512],
                                     start=st, stop=sp)
                    nc.tensor.matmul(PGB[:], HSTS[:, kt, ksl], GWS[:, kt, 512:600],
                                     start=st, stop=sp)
                GR = apool.tile([128, 601], BF16, tag=f"gr{k}")
                nc.scalar.activation(GR[:, 0:512], PGA[:], ACTF.Relu)
                nc.scalar.activation(GR[:, 512:600], PGB[:], ACTF.Relu)
                # LN stats in one DVE pass (two equal 300-col chunks so
                # bn_aggr's unweighted combine is exact)
                nc.vector.bn_stats(MV6[:, k, 0:6], GR[:, 0:300])
                nc.vector.bn_stats(MV6[:, k, 6:12], GR[:, 300:600])
                nc.vector.bn_aggr(MV[:, k, :], MV6[:, k, :])
                # mu column pairs with the classifier's -colsum row
                nc.vector.tensor_copy(GR[:, 600:601], MV[:, k, 0:1])
                GRS.append(GR)

            # rs = rsqrt(var + eps) via ACT Sqrt + DVE reciprocal.  The
            # pre-loop dummy Sqrt keeps the act table pinned, so this costs
            # one ~90ns ACT op instead of a per-iteration table reload.
            SD = stats.tile([128, jb], F32, tag="sd")
            nc.scalar.activation(SD[:], MV[:, :, 1], ACTF.Sqrt, bias=EPSC[:])
            Y = stats.tile([128, jb], F32, tag="y")
            nc.vector.reciprocal(Y[:], SD[:])
            W2 = stats.tile([128, jb], F32, tag="w2")
            nc.vector.tensor_mul(W2[:], WPRE[:], Y[:])
            W2F = stats.tile([128, jb * BC], BF16, tag="w2f")
            for k in range(jb):
                nc.vector.tensor_scalar(W2F[:, k * BC:(k + 1) * BC],
                                        SELS[:, k * BC:(k + 1) * BC],
                                        W2[:, k:k + 1], None, ALU.mult)

            # ---- aspects^T [601, BC] then classifier [3, BC] ----
            ASPT = sm_ps.tile([128, 5, BC], F32, tag="aspt")
            for hc, (hlo, hhi) in enumerate(HCH):
                sz = hhi - hlo
                for k in range(jb):
                    nc.tensor.matmul(ASPT[:sz, hc, :], GRS[k][:, hlo:hhi],
                                     W2F[:, k * BC:(k + 1) * BC],
                                     start=(k == 0), stop=(k == jb - 1))
            ASB = stats.tile([128, 5, BC], F32, tag="asb")
            nc.scalar.copy(ASB[:, 0:4, :], ASPT[:, 0:4, :])
            nc.vector.tensor_copy(ASB[0:89, 4, :], ASPT[0:89, 4, :])
            LG = sm_ps.tile([3, BC], F32, tag="lg")
            for hc, (hlo, hhi) in enumerate(HCH):
                sz = hhi - hlo
                nc.tensor.matmul(LG[:], CWGS[:sz, hc, :], ASB[:sz, hc, :],
                                 start=(hc == 0), stop=(hc == 4))
            OSB = stats.tile([3, BC], F32, tag="osb")
            nc.vector.tensor_add(OSB[:], LG[:], CLSB3[:])
            # issue from ACT: keeps the in-order SP queue free so the next
            # iteration's HSTS load can issue as soon as its WAR clears
            nc.scalar.dma_start(out_ap[:], OSB[:])

        UNROLL = 32
        if repeats <= UNROLL:
            hs = [load_hst() for _ in range(repeats)]
            for u in range(repeats):
                body(hs[u])
        else:
            # software-pipelined loads: the prologue fills all 8 buffers;
            # each body refills its buffer for the NEXT loop iteration as
            # soon as its guidance matmuls have consumed it, so the loop
            # back-edge never waits on a DMA
            assert repeats % UNROLL == 0, f"repeat count must divide {UNROLL}"
            hs = [load_hst() for _ in range(UNROLL)]
            with tc.For_i(0, repeats // UNROLL, 1, staggered_reset=True):
                for u in range(UNROLL):
                    body(hs[u])
                    refill_hst(hs[u])

    nc.compile()
    return nc


def prepare(inputs):
    """Host-side prep: pure index bookkeeping (unique-row packing, sample->
    core balancing, one-hot/selection masks) plus exact linear-algebra folds
    of the constant parameters.  All data-scale tensor arithmetic stays on
    device."""
    hs12 = np.asarray(inputs["hidden_states"])[12]              # [B, L, D]
    ts = np.asarray(inputs["token_starts"]).astype(np.int64)
    m = np.asarray(inputs["aspect_in_text_mask"], dtype=np.float32)
    gw = np.asarray(inputs["guid_W"], dtype=np.float32)[3]      # [D, H]
    gb = np.asarray(inputs["guid_b"], dtype=np.float32)[3]
    ln_g = np.asarray(inputs["ln_g"], dtype=np.float32)
    ln_b = np.asarray(inputs["ln_b"], dtype=np.float32)
    cls_W = np.asarray(inputs["cls_W"], dtype=np.float32)
    cls_b = np.asarray(inputs["cls_b"], dtype=np.float32)

    used_rows = [np.unique(ts[b][m[b] > 0]) for b in range(B)]
    ju = np.array([len(u) for u in used_rows])
    # LPT-balance samples across cores (exactly BC samples per core)
    order = np.argsort(-ju, kind="stable")
    cores = [[] for _ in range(N_CORES)]
    loads = np.zeros(N_CORES, np.int64)
    for b in order:
        cands = [c for c in range(N_CORES) if len(cores[c]) < BC]
        c = min(cands, key=lambda c: (loads[c], len(cores[c])))
        cores[c].append(int(b))
        loads[c] += ju[b]
    jb = max(1, int(np.ceil(loads.max() / 128)))
    jt = jb * 128
    has_bias = bool(np.any(gb != 0.0))

    cwg = ln_g[:, None] * cls_W                                  # [600, 3]
    cwg_full = np.zeros((640, 3), np.float32)
    cwg_full[:H] = cwg
    cwg_full[600] = -cwg.sum(0)
    clsb3 = np.tile((ln_b @ cls_W + cls_b)[:, None], (1, BC)).astype(np.float32)
    gw_b = np.ascontiguousarray(gw).astype(BF)

    in_maps = []
    for c in range(N_CORES):
        hst = np.zeros((D, jt), np.float32)
        wpre_flat = np.zeros(jt, np.float32)
        sel_flat = np.zeros((jt, BC), np.float32)
        j = 0
        for si, b in enumerate(cores[c]):
            rows = used_rows[b]
            msk = m[b] > 0
            cnt = np.zeros(L, np.float32)
            np.add.at(cnt, ts[b][msk], m[b][msk])
            n = len(rows)
            hst[:, j:j + n] = hs12[b][rows].T
            wpre_flat[j:j + n] = cnt[rows] / m[b].sum()
            sel_flat[j:j + n, si] = 1.0
            j += n
        hst[:, j:] = hst[:, 0:1]          # pad with a real column (w=0)
        # packed j -> (p = j % 128, k = j // 128)
        wpre = wpre_flat.reshape(jb, 128).T.copy()
        sels = sel_flat.reshape(jb, 128, BC).transpose(1, 0, 2).reshape(128, jb * BC).copy()
        im = dict(
            hst=np.ascontiguousarray(hst).astype(BF),
            gw=gw_b,
            sels=sels,
            wpre=wpre,
            cwg=cwg_full,
            clsb3=clsb3,
            epsc=np.full((128, 1), EPS, np.float32),
        )
        if has_bias:
            im["gbrow"] = gb[None, :].astype(BF)
            im["onesrow"] = np.ones((1, 128), BF)
        in_maps.append(im)
    return in_maps, cores, jb, has_bias


_PROGRAMS = {}


def kernel(**inputs):
    in_maps, cores, jb, has_bias = prepare(inputs)
    key = (jb, has_bias)
    nc = _PROGRAMS.get(key)
    if nc is None:
        nc = _PROGRAMS[key] = build_program(jb=jb, repeats=1, has_bias=has_bias)
    res = run_bass_kernel_spmd(nc, in_maps, list(range(N_CORES)), trace=False)
    out = np.zeros((B, 3), np.float32)
    for c in range(N_CORES):
        oc = np.asarray(res.results[c]["out"])   # [3, BC]
        for si, b in enumerate(cores[c]):
            out[b] = oc[:, si]
    return out


# revision 34
# speedup vs baseline: 1.1727x; 1.1398x over previous
"""Trainium2 Bass kernel for nn_BERT4GCN_53884659695997.

Mathematical reduction
----------------------
In the reference, ``feature`` is reassigned to ``LN(guidance)`` at the top of
every loop iteration, so the GCN block's output is never consumed; only the
last BERT layer's branch (index 3 -> hidden_states layer 12, which skips the
GCN block) reaches the output:

    t[b]      = LN(relu(hs[12,b][ts[b]] @ guid_W[3] + guid_b[3])) * ln_g + ln_b
    logits[b] = ((t[b] * m[b,:,None]).sum(0) / m[b].sum(0)) @ cls_W + cls_b

(verified numerically against the jax reference).

Row gathers commute with the row-wise ops (matmul-by-row / relu / LN), so the
gather+mask folds into per-source-row weights w[r] = sum_i m[i]*[ts[i]==r].
Only rows with w[r] != 0 reach the output (~47 unique masked rows per
sample).  The host does the index bookkeeping: it collects each sample's
unique masked rows, packs them contiguously across the 8 samples of a core
(~375 rows -> padded to JB*128 columns), and LPT-balances samples across the
8 cores so every core fits the same JB.  The packed rows are staged
transposed ([768, JB*128], bf16) so the device consumes them directly as
matmul stationary operands.

Device math per core (all tensor arithmetic on device, bf16 operands with
fp32 PSUM accumulation; output tolerance is 2e-2, measured ~4e-3):

    G   = HST^T @ GW (+ guid_b via a ones-row matmul, when nonzero)   # PE
    GR  = relu(G)                  # ACT, PSUM -> SBUF
    mu, var = bn_stats/bn_aggr(GR) # DVE, one pass
    rs  = rsqrt(var + eps)         # ACT Sqrt (pinned table) + DVE recip
    ASPT[h, s] = sum_j GRX[j, h] * (w_pre*rs*sel)[j, s]   # PE (col 600 = mu)
    logits^T   = CWG^T @ ASPT + CLSB                      # PE + DVE

LN folds into the classifier: the affine (ln_g, ln_b), the -mu correction
(via the extra mu column paired with a -sum(CWG) classifier row) and the
1/sum(m) normalization (folded into w_pre host-side) are all exact linear
algebra.  Sharding: data-parallel over batch B=64 -> 8 samples per core.

The repeat loop (measurement) unrolls 16 bodies inside a staggered-reset
tc.For_i with software-pipelined input loads: every body refills its HST
buffer right after its guidance matmuls consume it, so the loop back-edge
never waits on a DMA and the PE stream stays dense (HAM stays at full
clock).
"""

import numpy as np
import ml_dtypes
from contextlib import ExitStack

import concourse.bass as bass
import concourse.tile as tile
from concourse import bacc, mybir
from concourse.bass_utils import run_bass_kernel_spmd

F32 = mybir.dt.float32
BF16 = mybir.dt.bfloat16
AX = mybir.AxisListType
ALU = mybir.AluOpType
ACTF = mybir.ActivationFunctionType

N_CORES = 8
B = 64
BC = B // N_CORES
L = 256
D = 768
H = 600
KT = D // 128            # 6 contraction tiles
EPS = 1e-5
HCH = ((0, 128), (128, 256), (256, 384), (384, 512), (512, 601))
BF = ml_dtypes.bfloat16


def build_program(jb: int = 3, repeats: int = 1, has_bias: bool = False):
    jt = jb * 128
    nc = bacc.Bacc("TRN2", target_bir_lowering=False, debug=False,
                   num_devices=N_CORES)

    dr = {}
    def din(name, shape, dt=F32):
        dr[name] = nc.dram_tensor(name, list(shape), dt, kind="ExternalInput").ap()
    din("hst", (D, jt), BF16)        # packed gathered rows, transposed
    din("gw", (D, H), BF16)
    din("sels", (128, jb * BC))      # row-to-sample one-hot (packed layout)
    din("wpre", (128, jb))           # gather weights / sum(m), packed layout
    din("cwg", (640, 3))             # ln_g-folded cls_W; row 600 = -colsum
    din("clsb3", (3, BC))            # (ln_b @ cls_W + cls_b) replicated
    din("epsc", (128, 1))            # LN epsilon, ACT bias operand
    if has_bias:
        din("gbrow", (1, H), BF16)
        din("onesrow", (1, 128), BF16)
    out_ap = nc.dram_tensor("out", [3, BC], F32, kind="ExternalOutput").ap()

    with tile.TileContext(nc) as tc, ExitStack() as ctx:
        cpool = ctx.enter_context(tc.tile_pool(name="consts", bufs=1))
        hpool = ctx.enter_context(tc.tile_pool(name="stream", bufs=32))
        apool = ctx.enter_context(tc.tile_pool(name="act", bufs=2))
        stats = ctx.enter_context(tc.tile_pool(name="stats", bufs=2))
        pg_ps = ctx.enter_context(tc.tile_pool(name="pg", bufs=3, space="PSUM"))
        sm_ps = ctx.enter_context(tc.tile_pool(name="sm", bufs=1, space="PSUM"))

        # ---- constants (loaded once) ----
        GWS = cpool.tile([128, KT, H], BF16, tag="gws")
        nc.sync.dma_start(GWS[:], dr["gw"].rearrange("(kt p) n -> p kt n", p=128))
        SELS = cpool.tile([128, jb * BC], F32, tag="sels")
        nc.sync.dma_start(SELS[:], dr["sels"][:])
        WPRE = cpool.tile([128, jb], F32, tag="wpre")
        nc.sync.dma_start(WPRE[:], dr["wpre"][:])
        CWGS = cpool.tile([128, 5, 3], F32, tag="cwgs")
        nc.sync.dma_start(CWGS[:], dr["cwg"].rearrange("(c p) n -> p c n", p=128))
        CLSB3 = cpool.tile([3, BC], F32, tag="clsb3")
        nc.sync.dma_start(CLSB3[:], dr["clsb3"][:])
        EPSC = cpool.tile([128, 1], F32, tag="epsc")
        nc.sync.dma_start(EPSC[:], dr["epsc"][:])
        # dummy Sqrt before the loop pins the act-function table to
        # sqrt_and_others (contains Relu/Sqrt/Copy) so the in-loop Sqrt
        # never triggers a 1283ns table reload
        DUMS = cpool.tile([1, 1], F32, tag="dums")
        nc.scalar.activation(DUMS[:], EPSC[0:1, :], ACTF.Sqrt)
        if has_bias:
            GBROW = cpool.tile([1, H], BF16, tag="gbrow")
            nc.sync.dma_start(GBROW[:], dr["gbrow"][:])
            ONESR = cpool.tile([1, 128], BF16, tag="onesrow")
            nc.sync.dma_start(ONESR[:], dr["onesrow"][:])

        def load_hst():
            # one DMA for the packed gathered rows (the only big input)
            HSTS = hpool.tile([128, KT, jt], BF16, tag="hsts")
            refill_hst(HSTS)
            return HSTS

        def refill_hst(HSTS):
            nc.sync.dma_start(HSTS[:],
                              dr["hst"].rearrange("(kt p) j -> p kt j", p=128))

        def body(HSTS):
            MV6 = stats.tile([128, jb, 12], F32, tag="mv6")
            MV = stats.tile([128, jb, 2], F32, tag="mv")
            GRS = []
            for k in range(jb):
                ksl = slice(k * 128, (k + 1) * 128)
                # guidance matmul: out[j, n] accumulated over 6 k-tiles
                PGA = pg_ps.tile([128, 512], F32, tag="pga")
                PGB = pg_ps.tile([128, 88], F32, tag="pgb")
                if has_bias:
                    nc.tensor.matmul(PGA[:], ONESR[:], GBROW[:, 0:512],
                                     start=True, stop=False)
                    nc.tensor.matmul(PGB[:], ONESR[:], GBROW[:, 512:600],
                                     start=True, stop=False)
                for kt in range(KT):
                    st = (kt == 0) and not has_bias
                    sp = kt == KT - 1
                    nc.tensor.matmul(PGA[:], HSTS[:, kt, ksl], GWS[:, kt, 0:512],
                                     start=st, stop=sp)
                    nc.tensor.matmul(PGB[:], HSTS[:, kt, ksl], GWS[:, kt, 512:600],
                                     start=st, stop=sp)
                GR = apool.tile([128, 601], BF16, tag=f"gr{k}")
                nc.scalar.activation(GR[:, 0:512], PGA[:], ACTF.Relu)
                nc.scalar.activation(GR[:, 512:600], PGB[:], ACTF.Relu)
                # LN stats in one DVE pass (two equal 300-col chunks so
                # bn_aggr's unweighted combine is exact)
                nc.vector.bn_stats(MV6[:, k, 0:6], GR[:, 0:300])
                nc.vector.bn_stats(MV6[:, k, 6:12], GR[:, 300:600])
                nc.vector.bn_aggr(MV[:, k, :], MV6[:, k, :])
                # mu column pairs with the classifier's -colsum row
                nc.vector.tensor_copy(GR[:, 600:601], MV[:, k, 0:1])
                GRS.append(GR)

            # rs = rsqrt(var + eps) via ACT Sqrt + DVE reciprocal.  The
            # pre-loop dummy Sqrt keeps the act table pinned, so this costs
            # one ~90ns ACT op instead of a per-iteration table reload.
            SD = stats.tile([128, jb], F32, tag="sd")
            nc.scalar.activation(SD[:], MV[:, :, 1], ACTF.Sqrt, bias=EPSC[:])
            Y = stats.tile([128, jb], F32, tag="y")
            nc.vector.reciprocal(Y[:], SD[:])
            W2 = stats.tile([128, jb], F32, tag="w2")
            nc.vector.tensor_mul(W2[:], WPRE[:], Y[:])
            W2F = stats.tile([128, jb * BC], BF16, tag="w2f")
            for k in range(jb):
                nc.vector.tensor_scalar(W2F[:, k * BC:(k + 1) * BC],
                                        SELS[:, k * BC:(k + 1) * BC],
                                        W2[:, k:k + 1], None, ALU.mult)

            return GRS, W2F

        def body_tail(GRS, W2F):
            # ---- aspects^T [601, BC] then classifier [3, BC] ----
            # Emitted AFTER the next body's guidance matmuls (deferred tail):
            # PE's in-order stream then always has a full body of guidance
            # work ahead of these chain-dependent matmuls, so they never
            # stall the engine waiting on the relu->bn->rsqrt chain.
            ASPT = sm_ps.tile([128, 5, BC], F32, tag="aspt")
            for hc, (hlo, hhi) in enumerate(HCH):
                sz = hhi - hlo
                for k in range(jb):
                    nc.tensor.matmul(ASPT[:sz, hc, :], GRS[k][:, hlo:hhi],
                                     W2F[:, k * BC:(k + 1) * BC],
                                     start=(k == 0), stop=(k == jb - 1))
            ASB = stats.tile([128, 5, BC], F32, tag="asb")
            nc.scalar.copy(ASB[:, 0:4, :], ASPT[:, 0:4, :])
            nc.vector.tensor_copy(ASB[0:89, 4, :], ASPT[0:89, 4, :])
            LG = sm_ps.tile([3, BC], F32, tag="lg")
            for hc, (hlo, hhi) in enumerate(HCH):
                sz = hhi - hlo
                nc.tensor.matmul(LG[:], CWGS[:sz, hc, :], ASB[:sz, hc, :],
                                 start=(hc == 0), stop=(hc == 4))
            OSB = stats.tile([3, BC], F32, tag="osb")
            nc.vector.tensor_add(OSB[:], LG[:], CLSB3[:])
            # issue from ACT: keeps the in-order SP queue free so the next
            # iteration's HSTS load can issue as soon as its WAR clears
            nc.scalar.dma_start(out_ap[:], OSB[:])

        UNROLL = 32
        if repeats <= UNROLL:
            hs = [load_hst() for _ in range(repeats)]
            prev = None
            for u in range(repeats):
                cur = body(hs[u])
                if prev is not None:
                    body_tail(*prev)
                prev = cur
            body_tail(*prev)
        else:
            # software-pipelined loads: the prologue fills all buffers;
            # each body refills its buffer for the NEXT loop iteration as
            # soon as its guidance matmuls have consumed it, so the loop
            # back-edge never waits on a DMA
            assert repeats % UNROLL == 0, f"repeat count must divide {UNROLL}"
            hs = [load_hst() for _ in range(UNROLL)]
            with tc.For_i(0, repeats // UNROLL, 1, staggered_reset=True):
                prev = None
                for u in range(UNROLL):
                    cur = body(hs[u])
                    if prev is not None:
                        body_tail(*prev)
                    refill_hst(hs[u])
                    prev = cur
                body_tail(*prev)

    nc.compile()
    return nc


def prepare(inputs):
    """Host-side prep: pure index bookkeeping (unique-row packing, sample->
    core balancing, one-hot/selection masks) plus exact linear-algebra folds
    of the constant parameters.  All data-scale tensor arithmetic stays on
    device."""
    hs12 = np.asarray(inputs["hidden_states"])[12]              # [B, L, D]
    ts = np.asarray(inputs["token_starts"]).astype(np.int64)
    m = np.asarray(inputs["aspect_in_text_mask"], dtype=np.float32)
    gw = np.asarray(inputs["guid_W"], dtype=np.float32)[3]      # [D, H]
    gb = np.asarray(inputs["guid_b"], dtype=np.float32)[3]
    ln_g = np.asarray(inputs["ln_g"], dtype=np.float32)
    ln_b = np.asarray(inputs["ln_b"], dtype=np.float32)
    cls_W = np.asarray(inputs["cls_W"], dtype=np.float32)
    cls_b = np.asarray(inputs["cls_b"], dtype=np.float32)

    used_rows = [np.unique(ts[b][m[b] > 0]) for b in range(B)]
    ju = np.array([len(u) for u in used_rows])
    # LPT-balance samples across cores (exactly BC samples per core)
    order = np.argsort(-ju, kind="stable")
    cores = [[] for _ in range(N_CORES)]
    loads = np.zeros(N_CORES, np.int64)
    for b in order:
        cands = [c for c in range(N_CORES) if len(cores[c]) < BC]
        c = min(cands, key=lambda c: (loads[c], len(cores[c])))
        cores[c].append(int(b))
        loads[c] += ju[b]
    jb = max(1, int(np.ceil(loads.max() / 128)))
    jt = jb * 128
    has_bias = bool(np.any(gb != 0.0))

    cwg = ln_g[:, None] * cls_W                                  # [600, 3]
    cwg_full = np.zeros((640, 3), np.float32)
    cwg_full[:H] = cwg
    cwg_full[600] = -cwg.sum(0)
    clsb3 = np.tile((ln_b @ cls_W + cls_b)[:, None], (1, BC)).astype(np.float32)
    gw_b = np.ascontiguousarray(gw).astype(BF)

    in_maps = []
    for c in range(N_CORES):
        hst = np.zeros((D, jt), np.float32)
        wpre_flat = np.zeros(jt, np.float32)
        sel_flat = np.zeros((jt, BC), np.float32)
        j = 0
        for si, b in enumerate(cores[c]):
            rows = used_rows[b]
            msk = m[b] > 0
            cnt = np.zeros(L, np.float32)
            np.add.at(cnt, ts[b][msk], m[b][msk])
            n = len(rows)
            hst[:, j:j + n] = hs12[b][rows].T
            wpre_flat[j:j + n] = cnt[rows] / m[b].sum()
            sel_flat[j:j + n, si] = 1.0
            j += n
        hst[:, j:] = hst[:, 0:1]          # pad with a real column (w=0)
        # packed j -> (p = j % 128, k = j // 128)
        wpre = wpre_flat.reshape(jb, 128).T.copy()
        sels = sel_flat.reshape(jb, 128, BC).transpose(1, 0, 2).reshape(128, jb * BC).copy()
        im = dict(
            hst=np.ascontiguousarray(hst).astype(BF),
            gw=gw_b,
            sels=sels,
            wpre=wpre,
            cwg=cwg_full,
            clsb3=clsb3,
            epsc=np.full((128, 1), EPS, np.float32),
        )
        if has_bias:
            im["gbrow"] = gb[None, :].astype(BF)
            im["onesrow"] = np.ones((1, 128), BF)
        in_maps.append(im)
    return in_maps, cores, jb, has_bias


_PROGRAMS = {}


def kernel(**inputs):
    in_maps, cores, jb, has_bias = prepare(inputs)
    key = (jb, has_bias)
    nc = _PROGRAMS.get(key)
    if nc is None:
        nc = _PROGRAMS[key] = build_program(jb=jb, repeats=1, has_bias=has_bias)
    res = run_bass_kernel_spmd(nc, in_maps, list(range(N_CORES)), trace=False)
    out = np.zeros((B, 3), np.float32)
    for c in range(N_CORES):
        oc = np.asarray(res.results[c]["out"])   # [3, BC]
        for si, b in enumerate(cores[c]):
            out[b] = oc[:, si]
    return out


# revision 36
# speedup vs baseline: 1.2796x; 1.0911x over previous
"""Trainium2 Bass kernel for nn_BERT4GCN_53884659695997.

Mathematical reduction
----------------------
In the reference, ``feature`` is reassigned to ``LN(guidance)`` at the top of
every loop iteration, so the GCN block's output is never consumed; only the
last BERT layer's branch (index 3 -> hidden_states layer 12, which skips the
GCN block) reaches the output:

    t[b]      = LN(relu(hs[12,b][ts[b]] @ guid_W[3] + guid_b[3])) * ln_g + ln_b
    logits[b] = ((t[b] * m[b,:,None]).sum(0) / m[b].sum(0)) @ cls_W + cls_b

(verified numerically against the jax reference).

Row gathers commute with the row-wise ops (matmul-by-row / relu / LN), so the
gather+mask folds into per-source-row weights w[r] = sum_i m[i]*[ts[i]==r].
Only rows with w[r] != 0 reach the output (~47 unique masked rows per
sample).  The host does the index bookkeeping: it collects each sample's
unique masked rows, packs them contiguously across the 8 samples of a core
(~375 rows -> padded to JB*128 columns), and LPT-balances samples across the
8 cores so every core fits the same JB.  The packed rows are staged
transposed ([768, JB*128], bf16) so the device consumes them directly as
matmul stationary operands.

Device math per core (all tensor arithmetic on device, bf16 operands with
fp32 PSUM accumulation; output tolerance is 2e-2, measured ~4e-3):

    G   = HST^T @ GW (+ guid_b via a ones-row matmul, when nonzero)   # PE
    GR  = relu(G)                  # ACT, PSUM -> SBUF
    mu, var = bn_stats/bn_aggr(GR) # DVE, one pass
    rs  = rsqrt(var + eps)         # ACT Sqrt (pinned table) + DVE recip
    ASPT[h, s] = sum_j GRX[j, h] * (w_pre*rs*sel)[j, s]   # PE (col 600 = mu)
    logits^T   = CWG^T @ ASPT + CLSB                      # PE + DVE

LN folds into the classifier: the affine (ln_g, ln_b), the -mu correction
(via the extra mu column paired with a -sum(CWG) classifier row) and the
1/sum(m) normalization (folded into w_pre host-side) are all exact linear
algebra.  Sharding: data-parallel over batch B=64 -> 8 samples per core.

The repeat loop (measurement) unrolls 16 bodies inside a staggered-reset
tc.For_i with software-pipelined input loads: every body refills its HST
buffer right after its guidance matmuls consume it, so the loop back-edge
never waits on a DMA and the PE stream stays dense (HAM stays at full
clock).
"""

import numpy as np
import ml_dtypes
from contextlib import ExitStack

import concourse.bass as bass
import concourse.tile as tile
from concourse import bacc, mybir
from concourse.bass_utils import run_bass_kernel_spmd

F32 = mybir.dt.float32
BF16 = mybir.dt.bfloat16
AX = mybir.AxisListType
ALU = mybir.AluOpType
ACTF = mybir.ActivationFunctionType

N_CORES = 8
B = 64
BC = B // N_CORES
L = 256
D = 768
H = 600
KT = D // 128            # 6 contraction tiles
EPS = 1e-5
HCH = ((0, 128), (128, 256), (256, 384), (384, 512), (512, 601))
BF = ml_dtypes.bfloat16


def build_program(jb: int = 3, repeats: int = 1, has_bias: bool = False):
    jt = jb * 128
    nc = bacc.Bacc("TRN2", target_bir_lowering=False, debug=False,
                   num_devices=N_CORES)

    dr = {}
    def din(name, shape, dt=F32):
        dr[name] = nc.dram_tensor(name, list(shape), dt, kind="ExternalInput").ap()
    din("hst", (D, jt), BF16)        # packed gathered rows, transposed
    din("gw", (D, H), BF16)
    din("sels", (128, jb * BC))      # row-to-sample one-hot (packed layout)
    din("wpre", (128, jb))           # gather weights / sum(m), packed layout
    din("cwg", (640, 3))             # ln_g-folded cls_W; row 600 = -colsum
    din("clsb3", (3, BC))            # (ln_b @ cls_W + cls_b) replicated
    din("epsc", (128, 1))            # LN epsilon, ACT bias operand
    if has_bias:
        din("gbrow", (1, H), BF16)
        din("onesrow", (1, 128), BF16)
    out_ap = nc.dram_tensor("out", [3, BC], F32, kind="ExternalOutput").ap()

    with tile.TileContext(nc) as tc, ExitStack() as ctx:
        cpool = ctx.enter_context(tc.tile_pool(name="consts", bufs=1))
        hpool = ctx.enter_context(tc.tile_pool(name="stream", bufs=32))
        apool = ctx.enter_context(tc.tile_pool(name="act", bufs=2))
        stats = ctx.enter_context(tc.tile_pool(name="stats", bufs=2))
        pg_ps = ctx.enter_context(tc.tile_pool(name="pg", bufs=3, space="PSUM"))
        sm_ps = ctx.enter_context(tc.tile_pool(name="sm", bufs=1, space="PSUM"))

        # ---- constants (loaded once) ----
        GWS = cpool.tile([128, KT, H], BF16, tag="gws")
        nc.sync.dma_start(GWS[:], dr["gw"].rearrange("(kt p) n -> p kt n", p=128))
        SELS = cpool.tile([128, jb * BC], F32, tag="sels")
        nc.sync.dma_start(SELS[:], dr["sels"][:])
        WPRE = cpool.tile([128, jb], F32, tag="wpre")
        nc.sync.dma_start(WPRE[:], dr["wpre"][:])
        CWGS = cpool.tile([128, 5, 3], F32, tag="cwgs")
        nc.sync.dma_start(CWGS[:], dr["cwg"].rearrange("(c p) n -> p c n", p=128))
        CLSB3 = cpool.tile([3, BC], F32, tag="clsb3")
        nc.sync.dma_start(CLSB3[:], dr["clsb3"][:])
        EPSC = cpool.tile([128, 1], F32, tag="epsc")
        nc.sync.dma_start(EPSC[:], dr["epsc"][:])
        # dummy Sqrt before the loop pins the act-function table to
        # sqrt_and_others (contains Relu/Sqrt/Copy) so the in-loop Sqrt
        # never triggers a 1283ns table reload
        DUMS = cpool.tile([1, 1], F32, tag="dums")
        nc.scalar.activation(DUMS[:], EPSC[0:1, :], ACTF.Sqrt)
        if has_bias:
            GBROW = cpool.tile([1, H], BF16, tag="gbrow")
            nc.sync.dma_start(GBROW[:], dr["gbrow"][:])
            ONESR = cpool.tile([1, 128], BF16, tag="onesrow")
            nc.sync.dma_start(ONESR[:], dr["onesrow"][:])

        def load_hst():
            # one DMA for the packed gathered rows (the only big input)
            HSTS = hpool.tile([128, KT, jt], BF16, tag="hsts")
            refill_hst(HSTS)
            return HSTS

        def refill_hst(HSTS):
            nc.sync.dma_start(HSTS[:],
                              dr["hst"].rearrange("(kt p) j -> p kt j", p=128))

        def body(HSTS):
            MV6 = stats.tile([128, jb, 12], F32, tag="mv6")
            MV = stats.tile([128, jb, 2], F32, tag="mv")
            GRS = []
            for k in range(jb):
                ksl = slice(k * 128, (k + 1) * 128)
                # guidance matmul: out[j, n] accumulated over 6 k-tiles.
                # One two-bank PSUM tile; the 512-col group (bank 0) runs
                # fully, then the 88-col group (bank 1) — sequential groups
                # keep the zero-region checker happy, and the single tile
                # lets ONE relu cover all 600 columns.
                PG = pg_ps.tile([128, 600], F32, tag="pg")
                if has_bias:
                    nc.tensor.matmul(PG[:, 0:512], ONESR[:], GBROW[:, 0:512],
                                     start=True, stop=False)
                for kt in range(KT):
                    st = (kt == 0) and not has_bias
                    sp = kt == KT - 1
                    nc.tensor.matmul(PG[:, 0:512], HSTS[:, kt, ksl],
                                     GWS[:, kt, 0:512], start=st, stop=sp)
                if has_bias:
                    nc.tensor.matmul(PG[:, 512:600], ONESR[:], GBROW[:, 512:600],
                                     start=True, stop=False)
                for kt in range(KT):
                    st = (kt == 0) and not has_bias
                    sp = kt == KT - 1
                    nc.tensor.matmul(PG[:, 512:600], HSTS[:, kt, ksl],
                                     GWS[:, kt, 512:600], start=st, stop=sp)
                GR = apool.tile([128, 601], BF16, tag=f"gr{k}")
                nc.scalar.activation(GR[:, 0:600], PG[:], ACTF.Relu)
                # LN stats in one DVE pass (two equal 300-col chunks so
                # bn_aggr's unweighted combine is exact)
                nc.vector.bn_stats(MV6[:, k, 0:6], GR[:, 0:300])
                nc.vector.bn_stats(MV6[:, k, 6:12], GR[:, 300:600])
                nc.vector.bn_aggr(MV[:, k, :], MV6[:, k, :])
                # mu column pairs with the classifier's -colsum row
                nc.vector.tensor_copy(GR[:, 600:601], MV[:, k, 0:1])
                GRS.append(GR)

            # rs = rsqrt(var + eps) via ACT Sqrt + DVE reciprocal.  The
            # pre-loop dummy Sqrt keeps the act table pinned, so this costs
            # one ~90ns ACT op instead of a per-iteration table reload.
            SD = stats.tile([128, jb], F32, tag="sd")
            nc.scalar.activation(SD[:], MV[:, :, 1], ACTF.Sqrt, bias=EPSC[:])
            Y = stats.tile([128, jb], F32, tag="y")
            nc.vector.reciprocal(Y[:], SD[:])
            W2 = stats.tile([128, jb], F32, tag="w2")
            nc.vector.tensor_mul(W2[:], WPRE[:], Y[:])
            W2F = stats.tile([128, jb * BC], BF16, tag="w2f")
            for k in range(jb):
                nc.vector.tensor_scalar(W2F[:, k * BC:(k + 1) * BC],
                                        SELS[:, k * BC:(k + 1) * BC],
                                        W2[:, k:k + 1], None, ALU.mult)

            return GRS, W2F

        def body_tail(GRS, W2F):
            # ---- aspects^T [601, BC] then classifier [3, BC] ----
            # Emitted AFTER the next body's guidance matmuls (deferred tail):
            # PE's in-order stream then always has a full body of guidance
            # work ahead of these chain-dependent matmuls, so they never
            # stall the engine waiting on the relu->bn->rsqrt chain.
            ASPT = sm_ps.tile([128, 5, BC], F32, tag="aspt")
            for hc, (hlo, hhi) in enumerate(HCH):
                sz = hhi - hlo
                for k in range(jb):
                    nc.tensor.matmul(ASPT[:sz, hc, :], GRS[k][:, hlo:hhi],
                                     W2F[:, k * BC:(k + 1) * BC],
                                     start=(k == 0), stop=(k == jb - 1))
            ASB = stats.tile([128, 5, BC], F32, tag="asb")
            nc.scalar.copy(ASB[:, 0:4, :], ASPT[:, 0:4, :])
            nc.vector.tensor_copy(ASB[0:89, 4, :], ASPT[0:89, 4, :])
            LG = sm_ps.tile([3, BC], F32, tag="lg")
            for hc, (hlo, hhi) in enumerate(HCH):
                sz = hhi - hlo
                nc.tensor.matmul(LG[:], CWGS[:sz, hc, :], ASB[:sz, hc, :],
                                 start=(hc == 0), stop=(hc == 4))
            OSB = stats.tile([3, BC], F32, tag="osb")
            nc.vector.tensor_add(OSB[:], LG[:], CLSB3[:])
            # SP queue: with deferred tails each refill's deadline is a full
            # loop iteration away, so queueing the tiny out-DMA ahead of it
            # on in-order SP is harmless — and it frees ~0.5us/body of ACT
            # sequencer time (ACT is a near-binding chain engine)
            nc.sync.dma_start(out_ap[:], OSB[:])

        UNROLL = 32
        if repeats <= UNROLL:
            hs = [load_hst() for _ in range(repeats)]
            prev = None
            for u in range(repeats):
                cur = body(hs[u])
                if prev is not None:
                    body_tail(*prev)
                prev = cur
            body_tail(*prev)
        else:
            # software-pipelined loads: the prologue fills all buffers;
            # each body refills its buffer for the NEXT loop iteration as
            # soon as its guidance matmuls have consumed it, so the loop
            # back-edge never waits on a DMA
            assert repeats % UNROLL == 0, f"repeat count must divide {UNROLL}"
            hs = [load_hst() for _ in range(UNROLL)]
            with tc.For_i(0, repeats // UNROLL, 1, staggered_reset=True):
                prev = None
                for u in range(UNROLL):
                    cur = body(hs[u])
                    if prev is not None:
                        body_tail(*prev)
                    refill_hst(hs[u])
                    prev = cur
                body_tail(*prev)

    nc.compile()
    return nc


def prepare(inputs):
    """Host-side prep: pure index bookkeeping (unique-row packing, sample->
    core balancing, one-hot/selection masks) plus exact linear-algebra folds
    of the constant parameters.  All data-scale tensor arithmetic stays on
    device."""
    hs12 = np.asarray(inputs["hidden_states"])[12]              # [B, L, D]
    ts = np.asarray(inputs["token_starts"]).astype(np.int64)
    m = np.asarray(inputs["aspect_in_text_mask"], dtype=np.float32)
    gw = np.asarray(inputs["guid_W"], dtype=np.float32)[3]      # [D, H]
    gb = np.asarray(inputs["guid_b"], dtype=np.float32)[3]
    ln_g = np.asarray(inputs["ln_g"], dtype=np.float32)
    ln_b = np.asarray(inputs["ln_b"], dtype=np.float32)
    cls_W = np.asarray(inputs["cls_W"], dtype=np.float32)
    cls_b = np.asarray(inputs["cls_b"], dtype=np.float32)

    used_rows = [np.unique(ts[b][m[b] > 0]) for b in range(B)]
    ju = np.array([len(u) for u in used_rows])
    # LPT-balance samples across cores (exactly BC samples per core)
    order = np.argsort(-ju, kind="stable")
    cores = [[] for _ in range(N_CORES)]
    loads = np.zeros(N_CORES, np.int64)
    for b in order:
        cands = [c for c in range(N_CORES) if len(cores[c]) < BC]
        c = min(cands, key=lambda c: (loads[c], len(cores[c])))
        cores[c].append(int(b))
        loads[c] += ju[b]
    jb = max(1, int(np.ceil(loads.max() / 128)))
    jt = jb * 128
    has_bias = bool(np.any(gb != 0.0))

    cwg = ln_g[:, None] * cls_W                                  # [600, 3]
    cwg_full = np.zeros((640, 3), np.float32)
    cwg_full[:H] = cwg
    cwg_full[600] = -cwg.sum(0)
    clsb3 = np.tile((ln_b @ cls_W + cls_b)[:, None], (1, BC)).astype(np.float32)
    gw_b = np.ascontiguousarray(gw).astype(BF)

    in_maps = []
    for c in range(N_CORES):
        hst = np.zeros((D, jt), np.float32)
        wpre_flat = np.zeros(jt, np.float32)
        sel_flat = np.zeros((jt, BC), np.float32)
        j = 0
        for si, b in enumerate(cores[c]):
            rows = used_rows[b]
            msk = m[b] > 0
            cnt = np.zeros(L, np.float32)
            np.add.at(cnt, ts[b][msk], m[b][msk])
            n = len(rows)
            hst[:, j:j + n] = hs12[b][rows].T
            wpre_flat[j:j + n] = cnt[rows] / m[b].sum()
            sel_flat[j:j + n, si] = 1.0
            j += n
        hst[:, j:] = hst[:, 0:1]          # pad with a real column (w=0)
        # packed j -> (p = j % 128, k = j // 128)
        wpre = wpre_flat.reshape(jb, 128).T.copy()
        sels = sel_flat.reshape(jb, 128, BC).transpose(1, 0, 2).reshape(128, jb * BC).copy()
        im = dict(
            hst=np.ascontiguousarray(hst).astype(BF),
            gw=gw_b,
            sels=sels,
            wpre=wpre,
            cwg=cwg_full,
            clsb3=clsb3,
            epsc=np.full((128, 1), EPS, np.float32),
        )
        if has_bias:
            im["gbrow"] = gb[None, :].astype(BF)
            im["onesrow"] = np.ones((1, 128), BF)
        in_maps.append(im)
    return in_maps, cores, jb, has_bias


_PROGRAMS = {}


def kernel(**inputs):
    in_maps, cores, jb, has_bias = prepare(inputs)
    key = (jb, has_bias)
    nc = _PROGRAMS.get(key)
    if nc is None:
        nc = _PROGRAMS[key] = build_program(jb=jb, repeats=1, has_bias=has_bias)
    res = run_bass_kernel_spmd(nc, in_maps, list(range(N_CORES)), trace=False)
    out = np.zeros((B, 3), np.float32)
    for c in range(N_CORES):
        oc = np.asarray(res.results[c]["out"])   # [3, BC]
        for si, b in enumerate(cores[c]):
            out[b] = oc[:, si]
    return out
